# revision 1
# baseline (speedup 1.0000x reference)
"""Sparse masked attention layer for Trainium2, sharded over 8 NeuronCores.

Strategy
--------
The reference masks attention columns (keys) not in ``mask_ind`` with -inf
before softmax and zeroes rows (queries) not in ``mask_ind`` after softmax.
Both facts mean only the ~63% of token positions present in ``mask_ind``
participate at all: rows absent from the set produce exactly ``bproj`` in
the output.  So the host compacts each batch down to its kept token set,
the device runs *dense* attention on the compacted tokens (exactly equal
to the reference's masked softmax), and the host scatters results back,
filling non-kept rows with ``bproj``.

Sharding: core = (batch b, head-group g) -> 4 batches x 2 groups of 8
heads.  Each core computes q/k/v projections for its 8 heads from the
(replicated per-batch) compacted activations, attention per head, and its
partial contribution to the output projection (rows g*512:(g+1)*512 of
Wproj).  The two partials of a batch are summed on the host (D-sharded
matmul reduce) and bproj is added there.

Device layouts (per core, Cp = padded kept-token count):
  xT   [D, Cp]   compacted activations, transposed (host-side transpose)
  qkT  [128, 8, Cp] sbuf: chunks 0-3 = q features (512), 4-7 = k features
  v    [128, NC, 8, 65] sbuf: per c-chunk, per head: 64 v-features plus a
       "keep" column (1.0 for real tokens, 0.0 for padding).  The keep
       column makes the attention matmul compute the softmax denominator
       for free (row 64 of the AV output), with padded slots excluded.
  S^T  per head: psum [128 kept-k, q] = kT^T @ qT (K=64); exp via ACT with
       scale=1/8 fused.  Layout is transposed so P^T feeds the AV matmul
       directly as the moving operand (no transposes anywhere).
  attnT [64, 8, Cp] normalized attention output, transposed - exactly the
       lhsT layout the output projection needs.

All matmuls run in float32r (full-rate PE, ~1e-4 relative accuracy).
"""

import math

import numpy as np

B, C, D, H = 4, 2048, 1024, 16
HD = D // H          # 64
HPC = H // 2         # 8 heads per core
FQ = HPC * HD        # 512 per-core q/k/v feature count
N_CORES = 8

_NC_CACHE = {}


def _chunks(total, step):
    return [(i, min(step, total - i)) for i in range(0, total, step)]


def _build_nc(Cp):
    import concourse.mybir as mybir
    import concourse.tile as tile
    from concourse import bacc

    f32 = mybir.dt.float32
    f32r = mybir.dt.float32r
    Exp = mybir.ActivationFunctionType.Exp
    Ln = mybir.ActivationFunctionType.Ln

    NC = Cp // 128       # kept-token chunks of 128
    KD = D // 128        # 8 contraction chunks for the projections
    n512 = _chunks(Cp, 512)
    # q-dimension groups for attention: 512 wide (1 PSUM bank each)
    qgroups = _chunks(Cp, 512)
    qg_max = max(sz for _, sz in qgroups)

    nc = bacc.Bacc()
    xT = nc.dram_tensor("xT", [D, Cp], f32r, kind="ExternalInput")
    wqk = nc.dram_tensor("wqk", [D, 2 * FQ], f32r, kind="ExternalInput")
    bqk = nc.dram_tensor("bqk", [1, 2 * FQ], f32r, kind="ExternalInput")
    wv = nc.dram_tensor("wv", [D, FQ], f32r, kind="ExternalInput")
    bv = nc.dram_tensor("bv", [1, FQ], f32r, kind="ExternalInput")
    wp = nc.dram_tensor("wp", [FQ, D], f32r, kind="ExternalInput")
    keep = nc.dram_tensor("keep", [128, NC], f32, kind="ExternalInput")
    keepr = nc.dram_tensor("keepr", [128, NC], f32r, kind="ExternalInput")
    onesd = nc.dram_tensor("ones", [1, Cp], f32r, kind="ExternalInput")
    onesf = nc.dram_tensor("onesf", [1, 64], f32, kind="ExternalInput")
    outT = nc.dram_tensor("outT", [D, Cp], f32, kind="ExternalOutput")

    with tile.TileContext(nc) as tc:
        with tc.tile_pool(name="qkv", bufs=1) as p_qkv:
            qkT = p_qkv.tile([128, 8, Cp], f32r)
            vsb = p_qkv.tile([128, NC, HPC, HD + 1], f32r)

            # ---------------- phase A: projections ----------------
            with (
                tc.tile_pool(name="inp", bufs=1) as p_in,
                tc.tile_pool(name="psA", bufs=3, space="PSUM") as psA,
            ):
                xTs = p_in.tile([128, KD, Cp], f32r)
                wqks = p_in.tile([128, KD, 2 * FQ], f32r)
                wvs = p_in.tile([128, KD, FQ], f32r)
                for k in range(KD):
                    nc.sync.dma_start(wqks[:, k], wqk[k * 128:(k + 1) * 128, :])
                    nc.sync.dma_start(xTs[:, k], xT[k * 128:(k + 1) * 128, :])
                    nc.sync.dma_start(wvs[:, k], wv[k * 128:(k + 1) * 128, :])
                bqks = p_in.tile([1, 2 * FQ], f32r)
                nc.sync.dma_start(bqks[:], bqk[:])
                bvs = p_in.tile([1, FQ], f32r)
                nc.sync.dma_start(bvs[:], bv[:])
                keeps = p_in.tile([128, NC], f32)
                nc.sync.dma_start(keeps[:], keep[:])
                keeprs = p_in.tile([128, NC], f32r)
                nc.sync.dma_start(keeprs[:], keepr[:])
                ones = p_in.tile([1, Cp], f32r)
                nc.sync.dma_start(ones[:], onesd[:])

                # qkT[f, c] = (x @ Wqk + bqk)^T ; K=1 tail matmul adds the bias
                for m in range(8):
                    for n0, nsz in n512:
                        ps = psA.tile([128, 512], f32, tag="psA")
                        for k in range(KD):
                            nc.tensor.matmul(
                                ps[:, :nsz],
                                wqks[:, k, m * 128:(m + 1) * 128],
                                xTs[:, k, n0:n0 + nsz],
                                start=(k == 0), stop=False,
                            )
                        nc.tensor.matmul(
                            ps[:, :nsz],
                            bqks[0:1, m * 128:(m + 1) * 128],
                            ones[0:1, n0:n0 + nsz],
                            start=False, stop=True,
                        )
                        nc.vector.tensor_copy(qkT[:, m, n0:n0 + nsz], ps[:, :nsz])

                # v[c, f] = (x @ Wv + bv) * keep[c]; keep col = keep[c]
                for j in range(HPC):
                    nc.vector.tensor_copy(vsb[:, :, j, HD:HD + 1], keeprs[:])
                for c in range(NC):
                    ps = psA.tile([128, 512], f32, tag="psA")
                    for k in range(KD):
                        nc.tensor.matmul(
                            ps[:],
                            xTs[:, k, c * 128:(c + 1) * 128],
                            wvs[:, k, :],
                            start=(k == 0), stop=False,
                        )
                    nc.tensor.matmul(
                        ps[:], ones[0:1, c * 128:(c + 1) * 128], bvs[0:1, :],
                        start=False, stop=True,
                    )
                    nc.vector.tensor_scalar_mul(
                        vsb[:, c, :, 0:HD], ps[:], keeps[:, c:c + 1]
                    )

            # ---------------- phases B+C ----------------
            with (
                tc.tile_pool(name="att", bufs=2) as p_att,
                tc.tile_pool(name="pT", bufs=3) as p_pT,
                tc.tile_pool(name="attnT", bufs=1) as p_attnT,
                tc.tile_pool(name="wpp", bufs=1) as p_wp,
                tc.tile_pool(name="outs", bufs=3) as p_out,
            ):
                attnT = p_attnT.tile([128, HPC // 2, Cp], f32r)
                wps = p_wp.tile([128, HPC // 2, D], f32r)
                nc.sync.dma_start(wps[:], wp[:].rearrange("(c p) n -> p c n", p=128))
                onesfs = p_att.tile([1, 64], f32, tag="onesf", bufs=1)
                nc.sync.dma_start(onesfs[:], onesf[:])

                # phase B: attention.  Head pairs share the PE via row
                # tiling (even head in array rows 0-63, odd in 64-127).
                with (
                    tc.tile_pool(name="psS", bufs=2, space="PSUM") as psS,
                    tc.tile_pool(name="psAV", bufs=4, space="PSUM") as psAV,
                    tc.tile_pool(name="psBC", bufs=2, space="PSUM") as psBC,
                ):
                    for hp in range(4):
                        heads = (2 * hp, 2 * hp + 1)
                        for q0, qsz in qgroups:
                            avs = []
                            for hi, h in enumerate(heads):
                                avs.append(psAV.tile([65, qg_max], f32, tag="av",
                                                     name=f"av_{hp}_{q0}_{hi}"))
                            for kc in range(NC):
                                sss, pTs = [], []
                                for hi, h in enumerate(heads):
                                    lo = hi * 64
                                    ss = psS.tile([128, qg_max], f32, tag="ss")
                                    for s0, ssz in _chunks(qsz, 512):
                                        nc.tensor.matmul(
                                            ss[:, s0:s0 + ssz],
                                            qkT[lo:lo + 64, 4 + hp, kc * 128:(kc + 1) * 128],
                                            qkT[lo:lo + 64, hp, q0 + s0:q0 + s0 + ssz],
                                            start=True, stop=True,
                                        )
                                    sss.append(ss)
                                for hi, h in enumerate(heads):
                                    pT = p_pT.tile([128, qg_max], f32r, tag="pT")
                                    nc.scalar.activation(
                                        pT[:, :qsz], sss[hi][:, :qsz], Exp, scale=0.125
                                    )
                                    pTs.append(pT)
                                for hi, h in enumerate(heads):
                                    for s0, ssz in _chunks(qsz, 512):
                                        nc.tensor.matmul(
                                            avs[hi][:, s0:s0 + ssz],
                                            vsb[:, kc, h, :],
                                            pTs[hi][:, s0:s0 + ssz],
                                            start=(kc == 0), stop=(kc == NC - 1),
                                        )
                            for hi, h in enumerate(heads):
                                av = avs[hi]
                                # 1/denom on DVE (fast approx, ~18 bits), then
                                # broadcast across partitions via a K=1 PE
                                # outer product with a ones column.
                                dcp = p_att.tile([1, qg_max], f32, tag="dcp")
                                nc.vector.tensor_copy(dcp[0:1, :qsz],
                                                      av[64:65, :qsz])
                                rec = p_att.tile([1, qg_max], f32, tag="rec")
                                nc.vector.reciprocal_approx_fast(
                                    rec[0:1, :qsz], dcp[0:1, :qsz])
                                bcp = psBC.tile([64, qg_max], f32, tag="bc",
                                                name=f"bc_{hp}_{q0}_{hi}")
                                nc.tensor.matmul(bcp[:, :qsz], onesfs[0:1, :],
                                                 rec[0:1, :qsz],
                                                 start=True, stop=True)
                                bcs = p_att.tile([64, qg_max], f32, tag="bcs")
                                nc.scalar.copy(bcs[:, :qsz], bcp[:, :qsz])
                                lo = (h % 2) * 64
                                nc.vector.tensor_mul(
                                    attnT[lo:lo + 64, h // 2, q0:q0 + qsz],
                                    av[0:64, :qsz],
                                    bcs[:, :qsz],
                                )

                # phase C: output projection partial, transposed out
                with tc.tile_pool(name="psC", bufs=2, space="PSUM") as psC:
                    for m in range(8):
                        for n0, nsz in n512:
                            ps = psC.tile([128, 512], f32, tag="psC")
                            for j in range(HPC // 2):
                                nc.tensor.matmul(
                                    ps[:, :nsz],
                                    wps[:, j, m * 128:(m + 1) * 128],
                                    attnT[:, j, n0:n0 + nsz],
                                    start=(j == 0), stop=(j == HPC // 2 - 1),
                                )
                            st = p_out.tile([128, 512], f32, tag="st")
                            nc.vector.tensor_copy(st[:, :nsz], ps[:, :nsz])
                            nc.sync.dma_start(
                                outT[m * 128:(m + 1) * 128, n0:n0 + nsz], st[:, :nsz]
                            )

    nc.finalize()
    return nc


def _get_nc(Cp):
    if Cp not in _NC_CACHE:
        _NC_CACHE[Cp] = _build_nc(Cp)
    return _NC_CACHE[Cp]


def kernel(x, mask_ind, Wqkv, bqkv, Wproj, bproj, **_unused):
    from concourse.bass_utils import run_bass_kernel_spmd

    x = np.asarray(x, dtype=np.float32)
    mask_ind = np.asarray(mask_ind)
    Wqkv = np.asarray(Wqkv, dtype=np.float32)
    bqkv = np.asarray(bqkv, dtype=np.float32)
    Wproj = np.asarray(Wproj, dtype=np.float32)
    bproj = np.asarray(bproj, dtype=np.float32)

    # kept-token sets per batch (matches reference _keep_mask semantics)
    idx = []
    for b in range(B):
        mi = mask_ind[b]
        mi = mi[mi >= 0]
        mi = np.clip(mi, 0, C - 1)
        idx.append(np.unique(mi).astype(np.int64))
    nmax = max(len(u) for u in idx)
    Cp = max(128, ((nmax + 127) // 128) * 128)
    NC = Cp // 128

    nc = _get_nc(Cp)

    in_maps = []
    for core in range(N_CORES):
        b, g = core // 2, core % 2
        u = idx[b]
        n = len(u)
        xk = np.zeros((Cp, D), dtype=np.float32)
        xk[:n] = x[b, u]
        keep = np.zeros(Cp, dtype=np.float32)
        keep[:n] = 1.0
        qs, ks, vs = g * FQ, D + g * FQ, 2 * D + g * FQ
        wqk = np.concatenate(
            [Wqkv[:, qs:qs + FQ], Wqkv[:, ks:ks + FQ]], axis=1
        )
        bqk = np.concatenate([bqkv[qs:qs + FQ], bqkv[ks:ks + FQ]])
        in_maps.append({
            "xT": np.ascontiguousarray(xk.T),
            "wqk": np.ascontiguousarray(wqk),
            "bqk": bqk.reshape(1, -1),
            "wv": np.ascontiguousarray(Wqkv[:, vs:vs + FQ]),
            "bv": bqkv[vs:vs + FQ].reshape(1, -1).copy(),
            "wp": np.ascontiguousarray(Wproj[g * FQ:(g + 1) * FQ, :]),
            "keep": np.ascontiguousarray(keep.reshape(NC, 128).T),
            "keepr": np.ascontiguousarray(keep.reshape(NC, 128).T),
            "ones": np.ones((1, Cp), dtype=np.float32),
            "onesf": np.ones((1, 64), dtype=np.float32),
        })

    global _last_in_maps
    _last_in_maps = in_maps
    res = run_bass_kernel_spmd(nc, in_maps, core_ids=list(range(N_CORES)))

    out = np.broadcast_to(bproj, (B, C, D)).copy()
    for b in range(B):
        u = idx[b]
        n = len(u)
        comb = res.results[2 * b]["outT"] + res.results[2 * b + 1]["outT"]
        out[b, u] += comb.T[:n]
    return out



# revision 6
# speedup vs baseline: 2.2589x; 2.2589x over previous
"""Sparse masked attention layer for Trainium2, sharded over 8 NeuronCores.

Strategy
--------
Only token positions present in ``mask_ind`` participate (columns not in the
set get -inf pre-softmax; rows not in the set are zeroed post-softmax), so
the host compacts each batch to its kept token set, the device runs dense
attention on the compacted tokens, and the host scatters results back,
filling non-kept rows with ``bproj``.

Sharding: core = (batch b, head-group g) -> 4 batches x 2 groups of 8 heads.
Each core computes q/k/v projections for its 8 heads, attention per head,
and its partial contribution to the output projection (rows g*512:(g+1)*512
of Wproj).  The two partials of a batch are summed on the host.

Performance structure (v2):
  * all matmuls in bf16 (fp32 PSUM accumulate); the tiny reciprocal
    broadcast runs f32r (full-rate, exact fp32 bits).
  * S for a head pair is row-tiled on the PE (K=64 strips at partitions
    0-63 / 64-127, concurrent) into ONE 2-bank PSUM tile [128, 2, 512], so
    exp for both heads is a single merged ACT call (halves the ~352-cycle
    per-call overhead on the Scalar engine, the kernel's bottleneck).
  * exp output pT is bf16 and feeds the AV matmul directly; the AV "keep"
    column computes the softmax denominator for free.
  * software pipelining: S(kc+1) is emitted before AV(kc) so the PE works
    during the exp(kc) wait; V-projection chunks and the next head-pair's
    QK-projection groups are interleaved into phase B as PE filler to keep
    the HAM clock-gate warm (PE at 2.4 GHz, not 1.2).
  * q (moving) dimension trimmed to the true kept count Cq; the key side
    stays padded to Cp = ceil128 with keep-masking.
  * normalization denominator: row 64 of the AV output -> DVE reciprocal
    -> f32r K=1 PE broadcast -> DVE multiply (nothing on the Scalar engine
    except exp).
"""

import numpy as np

B, C, D, H = 4, 2048, 1024, 16
HD = D // H          # 64
HPC = H // 2         # 8 heads per core
FQ = HPC * HD        # 512 per-core q/k/v feature count
N_CORES = 8

_NC_CACHE = {}


def _chunks(total, step):
    return [(i, min(step, total - i)) for i in range(0, total, step)]


def _build_nc(Cp, Cq, has_bias):
    import concourse.mybir as mybir
    import concourse.tile as tile
    from concourse import bacc

    f32 = mybir.dt.float32
    f32r = mybir.dt.float32r
    bf16 = mybir.dt.bfloat16
    Exp = mybir.ActivationFunctionType.Exp

    NC = Cp // 128       # key chunks of 128
    KD = D // 128        # 8 contraction chunks for the projections
    nA = _chunks(Cp, 512)      # projection moving groups (full padded width)
    qgroups = _chunks(Cq, 512) # attention q groups (trimmed to real tokens)

    nc = bacc.Bacc()
    xT = nc.dram_tensor("xT", [D, Cp], bf16, kind="ExternalInput")
    wqk = nc.dram_tensor("wqk", [D, 2 * FQ], bf16, kind="ExternalInput")
    wv = nc.dram_tensor("wv", [D, FQ], bf16, kind="ExternalInput")
    wp = nc.dram_tensor("wp", [FQ, D], bf16, kind="ExternalInput")
    keep = nc.dram_tensor("keep", [128, NC], f32, kind="ExternalInput")
    onesf = nc.dram_tensor("onesf", [1, 64], f32r, kind="ExternalInput")
    if has_bias:
        bqkT = nc.dram_tensor("bqkT", [128, 8], f32, kind="ExternalInput")
        bvb = nc.dram_tensor("bvb", [128, FQ], f32, kind="ExternalInput")
    outT = nc.dram_tensor("outT", [D, Cq], f32, kind="ExternalOutput")

    with tile.TileContext(nc) as tc:
        with (
            tc.tile_pool(name="inp", bufs=1) as p_in,
            tc.tile_pool(name="big", bufs=1) as p_big,
            tc.tile_pool(name="pT", bufs=3) as p_pT,
            tc.tile_pool(name="att", bufs=2) as p_att,
            tc.tile_pool(name="outs", bufs=3) as p_out,
            tc.tile_pool(name="psA", bufs=1, space="PSUM") as psA,
            tc.tile_pool(name="psS", bufs=2, space="PSUM") as psS,
            tc.tile_pool(name="psAV", bufs=3, space="PSUM") as psAV,
        ):
            qkT = p_big.tile([128, 8, Cp], bf16)
            vsb = p_big.tile([128, NC, HPC, HD + 1], bf16)
            attnT = p_big.tile([128, HPC // 2, Cq], bf16)

            xTs = p_in.tile([128, KD, Cp], bf16)
            wqks = p_in.tile([128, KD, 2 * FQ], bf16)
            wvs = p_in.tile([128, KD, FQ], bf16)
            for k in range(KD):
                nc.sync.dma_start(xTs[:, k], xT[k * 128:(k + 1) * 128, :])
                nc.sync.dma_start(wqks[:, k], wqk[k * 128:(k + 1) * 128, :])
            for k in range(KD):
                nc.sync.dma_start(wvs[:, k], wv[k * 128:(k + 1) * 128, :])
            keeps = p_in.tile([128, NC], f32)
            nc.sync.dma_start(keeps[:], keep[:])
            onesfs = p_in.tile([1, 64], f32r)
            nc.sync.dma_start(onesfs[:], onesf[:])
            wps = p_in.tile([128, HPC // 2, D], bf16)
            nc.sync.dma_start(wps[:], wp[:].rearrange("(c p) n -> p c n", p=128))
            if has_bias:
                bqkTs = p_in.tile([128, 8], f32)
                nc.sync.dma_start(bqkTs[:], bqkT[:])
                bvbs = p_in.tile([128, FQ], f32)
                nc.sync.dma_start(bvbs[:], bvb[:])

            # qkT[f, c] = (x @ Wqk)^T for one 128-feature chunk m.
            def emit_qk_group(m, n0, nsz):
                ps = psA.tile([128, 512], f32, tag="psA")
                for k in range(KD):
                    nc.tensor.matmul(
                        ps[:, :nsz],
                        wqks[:, k, m * 128:(m + 1) * 128],
                        xTs[:, k, n0:n0 + nsz],
                        start=(k == 0), stop=(k == KD - 1),
                    )
                if has_bias:
                    nc.vector.tensor_scalar_add(
                        qkT[:, m, n0:n0 + nsz], ps[:, :nsz], bqkTs[:, m:m + 1]
                    )
                else:
                    nc.vector.tensor_copy(qkT[:, m, n0:n0 + nsz], ps[:, :nsz])

            # v[c-token, f] = (x @ Wv) * keep[c] for one 128-token chunk c.
            def emit_v_chunk(c):
                ps = psA.tile([128, 512], f32, tag="psA")
                for k in range(KD):
                    nc.tensor.matmul(
                        ps[:],
                        xTs[:, k, c * 128:(c + 1) * 128],
                        wvs[:, k, :],
                        start=(k == 0), stop=(k == KD - 1),
                    )
                if has_bias:
                    tmp = p_att.tile([128, FQ], f32, tag="vtmp")
                    nc.vector.tensor_add(tmp[:], ps[:], bvbs[:])
                    nc.vector.tensor_scalar_mul(
                        vsb[:, c, :, 0:HD], tmp[:], keeps[:, c:c + 1]
                    )
                else:
                    nc.vector.tensor_scalar_mul(
                        vsb[:, c, :, 0:HD], ps[:], keeps[:, c:c + 1]
                    )

            # keep columns (softmax denominator rides row 64 of AV output)
            for j in range(HPC):
                nc.vector.tensor_copy(vsb[:, :, j, HD:HD + 1], keeps[:])

            # prefix: q/k features for head pair 0, then the first v chunk
            for m in (0, 4):
                for n0, nsz in nA:
                    emit_qk_group(m, n0, nsz)
            emit_v_chunk(0)

            # ---------------- attention, head pair hp ----------------
            def emit_S(hp, q0, qsz, kc, ss):
                for hi in range(2):
                    lo = hi * 64
                    nc.tensor.matmul(
                        ss[:, hi, :qsz],
                        qkT[lo:lo + 64, 4 + hp, kc * 128:(kc + 1) * 128],
                        qkT[lo:lo + 64, hp, q0:q0 + qsz],
                        start=True, stop=True,
                    )

            for hp in range(4):
                # PE filler to emit inside this hp's ACT-bound kc loop:
                # hp<3 pre-computes the next pair's q/k projections.
                fillers = []
                if hp < 3:
                    for m in (hp + 1, 4 + hp + 1):
                        for n0, nsz in nA:
                            fillers.append((emit_qk_group, (m, n0, nsz)))
                for gi, (q0, qsz) in enumerate(qgroups):
                    avs = [
                        psAV.tile([65, 512], f32, tag="av",
                                  name=f"av_{hp}_{q0}_{hi}")
                        for hi in range(2)
                    ]
                    ss_cur = psS.tile([128, 2, 512], f32, tag="ss")
                    emit_S(hp, q0, qsz, 0, ss_cur)
                    for kc in range(NC):
                        # exp of both heads in one merged ACT call
                        pT = p_pT.tile([128, 2, 512], bf16, tag="pT")
                        nc.scalar.activation(
                            pT[:, :, :qsz], ss_cur[:, :, :qsz], Exp, scale=0.125
                        )
                        # pipeline: S(kc+1) ahead of AV(kc)
                        if kc + 1 < NC:
                            ss_nxt = psS.tile([128, 2, 512], f32, tag="ss")
                            emit_S(hp, q0, qsz, kc + 1, ss_nxt)
                        # PE filler while ACT grinds
                        if hp == 0 and gi == 0:
                            if kc + 1 < NC:
                                emit_v_chunk(kc + 1)
                        elif fillers and kc % 4 == 1:
                            f, args = fillers.pop(0)
                            f(*args)
                        for hi in range(2):
                            nc.tensor.matmul(
                                avs[hi][:, :qsz],
                                vsb[:, kc, 2 * hp + hi, :],
                                pT[:, hi, :qsz],
                                start=(kc == 0), stop=(kc == NC - 1),
                            )
                        if kc + 1 < NC:
                            ss_cur = ss_nxt
                    # drain leftover filler at the end of the q-group
                    if gi == len(qgroups) - 1:
                        while fillers:
                            f, args = fillers.pop(0)
                            f(*args)
                    # normalize: out = av[0:64] / av[64]
                    for hi in range(2):
                        av = avs[hi]
                        dcp = p_att.tile([1, 512], f32, tag="dcp")
                        nc.vector.tensor_copy(dcp[0:1, :qsz], av[64:65, :qsz])
                        rec = p_att.tile([1, 512], f32, tag="rec")
                        nc.vector.reciprocal_approx_fast(
                            rec[0:1, :qsz], dcp[0:1, :qsz])
                        recr = p_att.tile([1, 512], f32r, tag="recr")
                        nc.vector.tensor_copy(recr[0:1, :qsz], rec[0:1, :qsz])
                        bcp = psA.tile([128, 512], f32, tag="psA",
                                       name=f"bc_{hp}_{q0}_{hi}")
                        nc.tensor.matmul(bcp[0:64, :qsz], onesfs[0:1, :],
                                         recr[0:1, :qsz],
                                         start=True, stop=True)
                        bcs = p_att.tile([64, 512], f32, tag="bcs")
                        nc.vector.tensor_copy(bcs[:, :qsz], bcp[0:64, :qsz])
                        lo = hi * 64
                        nc.vector.tensor_mul(
                            attnT[lo:lo + 64, hp, q0:q0 + qsz],
                            av[0:64, :qsz],
                            bcs[:, :qsz],
                        )

            # ---------------- output projection partial, transposed out
            for m in range(8):
                for n0, nsz in qgroups:
                    ps = psS.tile([128, 2, 512], f32, tag="ss")
                    for j in range(HPC // 2):
                        nc.tensor.matmul(
                            ps[:, 0, :nsz],
                            wps[:, j, m * 128:(m + 1) * 128],
                            attnT[:, j, n0:n0 + nsz],
                            start=(j == 0), stop=(j == HPC // 2 - 1),
                        )
                    st = p_out.tile([128, 512], f32, tag="st")
                    nc.vector.tensor_copy(st[:, :nsz], ps[:, 0, :nsz])
                    nc.sync.dma_start(
                        outT[m * 128:(m + 1) * 128, n0:n0 + nsz], st[:, :nsz]
                    )

    nc.finalize()
    return nc


def _get_nc(Cp, Cq, has_bias):
    key = (Cp, Cq, has_bias)
    if key not in _NC_CACHE:
        _NC_CACHE[key] = _build_nc(Cp, Cq, has_bias)
    return _NC_CACHE[key]


def kernel(x, mask_ind, Wqkv, bqkv, Wproj, bproj, **_unused):
    import ml_dtypes
    from concourse.bass_utils import run_bass_kernel_spmd

    bf = ml_dtypes.bfloat16
    x = np.asarray(x, dtype=np.float32)
    mask_ind = np.asarray(mask_ind)
    Wqkv = np.asarray(Wqkv, dtype=np.float32)
    bqkv = np.asarray(bqkv, dtype=np.float32)
    Wproj = np.asarray(Wproj, dtype=np.float32)
    bproj = np.asarray(bproj, dtype=np.float32)

    # kept-token sets per batch (matches reference _keep_mask semantics)
    idx = []
    for b in range(B):
        mi = mask_ind[b]
        mi = mi[mi >= 0]
        mi = np.clip(mi, 0, C - 1)
        idx.append(np.unique(mi).astype(np.int64))
    Cq = max(128, max(len(u) for u in idx))
    Cp = ((Cq + 127) // 128) * 128
    NC = Cp // 128
    has_bias = bool(np.any(bqkv))

    nc = _get_nc(Cp, Cq, has_bias)

    in_maps = []
    for core in range(N_CORES):
        b, g = core // 2, core % 2
        u = idx[b]
        n = len(u)
        xk = np.zeros((Cp, D), dtype=np.float32)
        xk[:n] = x[b, u]
        keep = np.zeros(Cp, dtype=np.float32)
        keep[:n] = 1.0
        qs, ks, vs = g * FQ, D + g * FQ, 2 * D + g * FQ
        wqk = np.concatenate(
            [Wqkv[:, qs:qs + FQ], Wqkv[:, ks:ks + FQ]], axis=1
        )
        im = {
            "xT": np.ascontiguousarray(xk.T).astype(bf),
            "wqk": np.ascontiguousarray(wqk).astype(bf),
            "wv": np.ascontiguousarray(Wqkv[:, vs:vs + FQ]).astype(bf),
            "wp": np.ascontiguousarray(Wproj[g * FQ:(g + 1) * FQ, :]).astype(bf),
            "keep": np.ascontiguousarray(keep.reshape(NC, 128).T),
            "onesf": np.ones((1, 64), dtype=np.float32),
        }
        if has_bias:
            bqk = np.concatenate([bqkv[qs:qs + FQ], bqkv[ks:ks + FQ]])
            im["bqkT"] = np.ascontiguousarray(bqk.reshape(8, 128).T)
            im["bvb"] = np.broadcast_to(
                bqkv[vs:vs + FQ], (128, FQ)).astype(np.float32).copy()
        in_maps.append(im)

    global _last_in_maps
    _last_in_maps = in_maps
    res = run_bass_kernel_spmd(nc, in_maps, core_ids=list(range(N_CORES)))

    out = np.broadcast_to(bproj, (B, C, D)).copy()
    for b in range(B):
        u = idx[b]
        n = len(u)
        comb = res.results[2 * b]["outT"] + res.results[2 * b + 1]["outT"]
        out[b, u] += comb.T[:n]
    return out


# revision 12
# speedup vs baseline: 2.4493x; 1.0843x over previous
"""Sparse masked attention layer for Trainium2, sharded over 8 NeuronCores.

Strategy
--------
Only token positions present in ``mask_ind`` participate (columns not in the
set get -inf pre-softmax; rows not in the set are zeroed post-softmax), so
the host compacts each batch to its kept token set, the device runs dense
attention on the compacted tokens, and the host scatters results back,
filling non-kept rows with ``bproj``.

Sharding: core = (batch b, head-group g) -> 4 batches x 2 groups of 8 heads.
Each core computes q/k/v projections for its 8 heads, attention per head,
and its partial contribution to the output projection (rows g*512:(g+1)*512
of Wproj).  The two partials of a batch are summed on the host.

Performance structure (v3):
  * all matmuls in bf16 (fp32 PSUM accumulate); the tiny reciprocal
    broadcast runs f32r (full-rate, exact fp32 bits).
  * S for a head pair is row-tiled on the PE (K=64 strips at partitions
    0-63 / 64-127, concurrent) into ONE 2-bank PSUM tile [128, 2, 512], so
    exp for both heads is a single merged ACT call (halves the ~352-cycle
    per-call overhead on the Scalar engine, the kernel's bottleneck).
  * exp output pT is bf16 and feeds the AV matmul directly; the AV "keep"
    column computes the softmax denominator for free.
  * software pipelining: S(kc+1) is emitted before AV(kc) so the PE works
    during the exp(kc) wait; V-projection chunks and the next head-pair's
    QK-projection groups are interleaved into the kc loop as PE filler to
    keep the HAM clock-gate warm (PE at 2.4 GHz, not 1.2).
  * softmax normalization is itself pipelined: the AV accumulator (with the
    denominator in row 64) is copied PSUM->SBUF right after the last AV,
    and the reciprocal / broadcast / multiply chain runs inside the NEXT
    q-group's kc loop so neither the PE nor ACT ever waits on it.  The last
    group's normalization hides under the first output-projection columns.
  * q (moving) dimension trimmed to the true kept count Cq; outputs are
    staged per 128-feature chunk and shipped with one DMA each on the
    GpSimd queue (input DMAs split between Sync and GpSimd queues).
"""

import numpy as np

B, C, D, H = 4, 2048, 1024, 16
HD = D // H          # 64
HPC = H // 2         # 8 heads per core
FQ = HPC * HD        # 512 per-core q/k/v feature count
N_CORES = 8

_NC_CACHE = {}


def _chunks(total, step):
    return [(i, min(step, total - i)) for i in range(0, total, step)]


def _build_nc(Cp, Cq, has_bias):
    import concourse.mybir as mybir
    import concourse.tile as tile
    from concourse import bacc

    f32 = mybir.dt.float32
    f32r = mybir.dt.float32r
    bf16 = mybir.dt.bfloat16
    Exp = mybir.ActivationFunctionType.Exp

    NC = Cp // 128       # key chunks of 128
    KD = D // 128        # 8 contraction chunks for the projections
    nA = _chunks(Cp, 512)      # projection moving groups (full padded width)
    qgroups = _chunks(Cq, 512) # attention q groups (trimmed to real tokens)
    NG = len(qgroups)

    nc = bacc.Bacc()
    xT = nc.dram_tensor("xT", [D, Cp], bf16, kind="ExternalInput")
    wqk = nc.dram_tensor("wqk", [D, 2 * FQ], bf16, kind="ExternalInput")
    wv = nc.dram_tensor("wv", [D, FQ], bf16, kind="ExternalInput")
    wp = nc.dram_tensor("wp", [FQ, D], bf16, kind="ExternalInput")
    keep = nc.dram_tensor("keep", [128, NC], f32, kind="ExternalInput")
    onesf = nc.dram_tensor("onesf", [1, 64], f32r, kind="ExternalInput")
    if has_bias:
        bqkT = nc.dram_tensor("bqkT", [128, 8], f32, kind="ExternalInput")
        bvb = nc.dram_tensor("bvb", [128, FQ], f32, kind="ExternalInput")
    outT = nc.dram_tensor("outT", [D, Cq], f32, kind="ExternalOutput")

    with tile.TileContext(nc) as tc:
        with (
            tc.tile_pool(name="inp", bufs=1) as p_in,
            tc.tile_pool(name="big", bufs=1) as p_big,
            tc.tile_pool(name="pT", bufs=3) as p_pT,
            tc.tile_pool(name="att", bufs=2) as p_att,
            tc.tile_pool(name="outs", bufs=2) as p_out,
            tc.tile_pool(name="psA", bufs=1, space="PSUM") as psA,
            tc.tile_pool(name="psS", bufs=2, space="PSUM") as psS,
            tc.tile_pool(name="psAV", bufs=3, space="PSUM") as psAV,
        ):
            qkT = p_big.tile([128, 8, Cp], bf16)
            vsb = p_big.tile([128, NC, HPC, HD + 1], bf16)
            attnT = p_big.tile([128, HPC // 2, Cq], bf16)

            xTs = p_in.tile([128, KD, Cp], bf16)
            wqks = p_in.tile([128, KD, 2 * FQ], bf16)
            wvs = p_in.tile([128, KD, FQ], bf16)
            for k in range(KD):
                nc.sync.dma_start(xTs[:, k], xT[k * 128:(k + 1) * 128, :])
                nc.sync.dma_start(wqks[:, k], wqk[k * 128:(k + 1) * 128, :])
            wvr = wv[:].rearrange("(k p) n -> p k n", p=128)
            nc.sync.dma_start(wvs[:, 0:4], wvr[:, 0:4])
            nc.sync.dma_start(wvs[:, 4:8], wvr[:, 4:8])
            keeps = p_in.tile([128, NC], f32)
            nc.sync.dma_start(keeps[:], keep[:])
            onesfs = p_in.tile([1, 64], f32r)
            nc.sync.dma_start(onesfs[:], onesf[:])
            wps = p_in.tile([128, HPC // 2, D], bf16)
            nc.sync.dma_start(wps[:], wp[:].rearrange("(c p) n -> p c n", p=128))
            if has_bias:
                bqkTs = p_in.tile([128, 8], f32)
                nc.sync.dma_start(bqkTs[:], bqkT[:])
                bvbs = p_in.tile([128, FQ], f32)
                nc.sync.dma_start(bvbs[:], bvb[:])

            # qkT[f, c] = (x @ Wqk)^T for one 128-feature chunk m.
            def emit_qk_group(m, n0, nsz):
                ps = psA.tile([128, 512], f32, tag="psA")
                for k in range(KD):
                    nc.tensor.matmul(
                        ps[:, :nsz],
                        wqks[:, k, m * 128:(m + 1) * 128],
                        xTs[:, k, n0:n0 + nsz],
                        start=(k == 0), stop=(k == KD - 1),
                    )
                if has_bias:
                    nc.vector.tensor_scalar_add(
                        qkT[:, m, n0:n0 + nsz], ps[:, :nsz], bqkTs[:, m:m + 1]
                    )
                else:
                    nc.vector.tensor_copy(qkT[:, m, n0:n0 + nsz], ps[:, :nsz])

            # v[c-token, f] = (x @ Wv) * keep[c] for one 128-token chunk c.
            def emit_v_chunk(c):
                ps = psA.tile([128, 512], f32, tag="psA")
                for k in range(KD):
                    nc.tensor.matmul(
                        ps[:],
                        xTs[:, k, c * 128:(c + 1) * 128],
                        wvs[:, k, :],
                        start=(k == 0), stop=(k == KD - 1),
                    )
                if has_bias:
                    tmp = p_att.tile([128, FQ], f32, tag="vtmp")
                    nc.vector.tensor_add(tmp[:], ps[:], bvbs[:])
                    nc.vector.tensor_scalar_mul(
                        vsb[:, c, :, 0:HD], tmp[:], keeps[:, c:c + 1]
                    )
                else:
                    nc.vector.tensor_scalar_mul(
                        vsb[:, c, :, 0:HD], ps[:], keeps[:, c:c + 1]
                    )

            # keep columns (softmax denominator rides row 64 of AV output)
            for j in range(HPC):
                nc.vector.tensor_copy(vsb[:, :, j, HD:HD + 1], keeps[:])

            # prefix: q/k features for head pair 0, then the first v chunk
            for m in (0, 4):
                for n0, nsz in nA:
                    emit_qk_group(m, n0, nsz)
            emit_v_chunk(0)

            # ---------------- attention ----------------
            def emit_S(hp, q0, qsz, kc, ss):
                for hi in range(2):
                    lo = hi * 64
                    nc.tensor.matmul(
                        ss[:, hi, :qsz],
                        qkT[lo:lo + 64, 4 + hp, kc * 128:(kc + 1) * 128],
                        qkT[lo:lo + 64, hp, q0:q0 + qsz],
                        start=True, stop=True,
                    )

            # deferred normalization: out = av[0:64] / av[64]
            def norm_dve(prev):
                avs, hp, q0, qsz = prev
                st = []
                for hi in range(2):
                    dcp = p_att.tile([1, 512], f32, tag=f"dcp{hi}")
                    nc.vector.tensor_copy(dcp[0:1, :qsz], avs[hi][64:65, :qsz])
                    avsb = p_att.tile([64, 512], f32, tag=f"avsb{hi}")
                    nc.vector.tensor_copy(avsb[:, :qsz], avs[hi][0:64, :qsz])
                    rec = p_att.tile([1, 512], f32, tag=f"rec{hi}")
                    nc.vector.reciprocal_approx_fast(
                        rec[0:1, :qsz], dcp[0:1, :qsz])
                    recr = p_att.tile([1, 512], f32r, tag=f"recr{hi}")
                    nc.vector.tensor_copy(recr[0:1, :qsz], rec[0:1, :qsz])
                    st.append((avsb, recr))
                return st

            def norm_head(prev, st, hi):
                _, hp, q0, qsz = prev
                avsb, recr = st[hi]
                bcp = psA.tile([128, 512], f32, tag="psA")
                nc.tensor.matmul(bcp[0:64, :qsz], onesfs[0:1, :],
                                 recr[0:1, :qsz], start=True, stop=True)
                bcs = p_att.tile([64, 512], f32, tag=f"bcs{hi}")
                nc.vector.tensor_copy(bcs[:, :qsz], bcp[0:64, :qsz])
                lo = hi * 64
                nc.vector.tensor_mul(
                    attnT[lo:lo + 64, hp, q0:q0 + qsz],
                    avsb[:, :qsz],
                    bcs[:, :qsz],
                )

            groups = [(hp, q0, qsz) for hp in range(4) for q0, qsz in qgroups]
            fill_by_hp = {hp: [] for hp in range(4)}
            for hp in range(3):
                for m in (hp + 1, 4 + hp + 1):
                    for n0, nsz in nA:
                        fill_by_hp[hp].append((m, n0, nsz))

            prev = None      # finished group awaiting normalization
            prev_st = None
            ndone = 0
            for g, (hp, q0, qsz) in enumerate(groups):
                # normalization DVE chain of the previous group is emitted
                # BEFORE this group's PSUM accumulators are allocated, so
                # the pool recycle sees the copies as registered consumers.
                if prev is not None:
                    prev_st = norm_dve(prev)
                    ndone = 0
                avs = [
                    psAV.tile([65, 512], f32, tag="av", name=f"av_{g}_{hi}")
                    for hi in range(2)
                ]
                ss_cur = psS.tile([128, 2, 512], f32, tag="ss")
                emit_S(hp, q0, qsz, 0, ss_cur)
                for kc in range(NC):
                    pT = p_pT.tile([128, 2, 512], bf16, tag="pT")
                    nc.scalar.activation(
                        pT[:, :, :qsz], ss_cur[:, :, :qsz], Exp, scale=0.125
                    )
                    if kc + 1 < NC:
                        ss_nxt = psS.tile([128, 2, 512], f32, tag="ss")
                        emit_S(hp, q0, qsz, kc + 1, ss_nxt)
                    if g == 0:
                        if kc + 1 < NC:
                            emit_v_chunk(kc + 1)
                    elif kc % 3 == 2 and fill_by_hp[hp]:
                        emit_qk_group(*fill_by_hp[hp].pop(0))
                    for hi in range(2):
                        nc.tensor.matmul(
                            avs[hi][:, :qsz],
                            vsb[:, kc, 2 * hp + hi, :],
                            pT[:, hi, :qsz],
                            start=(kc == 0), stop=(kc == NC - 1),
                        )
                    if prev is not None and kc in (1, 2):
                        norm_head(prev, prev_st, kc - 1)
                        ndone = kc
                    if kc + 1 < NC:
                        ss_cur = ss_nxt
                # drain: unfinished normalization (NC < 3) and leftover
                # projection filler on this head pair's last group
                if prev is not None:
                    for hi in range(ndone, 2):
                        norm_head(prev, prev_st, hi)
                if g % NG == NG - 1:
                    while fill_by_hp[hp]:
                        emit_qk_group(*fill_by_hp[hp].pop(0))
                prev = (avs, hp, q0, qsz)

            # ---------------- output projection partial, transposed out
            # last group's normalization hides under the first C columns
            last_st = norm_dve(prev)
            for m in range(8):
                stq = p_out.tile([128, Cq], f32, tag="st")
                for gi, (n0, nsz) in enumerate(qgroups):
                    ps = psS.tile([128, 2, 512], f32, tag="ss")
                    for j in range(HPC // 2):
                        nc.tensor.matmul(
                            ps[:, 0, :nsz],
                            wps[:, j, m * 128:(m + 1) * 128],
                            attnT[:, j, n0:n0 + nsz],
                            start=(j == 0), stop=(j == HPC // 2 - 1),
                        )
                    if m == 0 and gi < 2:
                        norm_head(prev, last_st, gi)
                    nc.vector.tensor_copy(stq[:, n0:n0 + nsz], ps[:, 0, :nsz])
                nc.sync.dma_start(outT[m * 128:(m + 1) * 128, :], stq[:, :])

    nc.finalize()
    return nc


def _get_nc(Cp, Cq, has_bias):
    key = (Cp, Cq, has_bias)
    if key not in _NC_CACHE:
        _NC_CACHE[key] = _build_nc(Cp, Cq, has_bias)
    return _NC_CACHE[key]


def kernel(x, mask_ind, Wqkv, bqkv, Wproj, bproj, **_unused):
    import ml_dtypes
    from concourse.bass_utils import run_bass_kernel_spmd

    bf = ml_dtypes.bfloat16
    x = np.asarray(x, dtype=np.float32)
    mask_ind = np.asarray(mask_ind)
    Wqkv = np.asarray(Wqkv, dtype=np.float32)
    bqkv = np.asarray(bqkv, dtype=np.float32)
    Wproj = np.asarray(Wproj, dtype=np.float32)
    bproj = np.asarray(bproj, dtype=np.float32)

    # kept-token sets per batch (matches reference _keep_mask semantics)
    idx = []
    for b in range(B):
        mi = mask_ind[b]
        mi = mi[mi >= 0]
        mi = np.clip(mi, 0, C - 1)
        idx.append(np.unique(mi).astype(np.int64))
    Cq = max(128, max(len(u) for u in idx))
    Cp = ((Cq + 127) // 128) * 128
    NC = Cp // 128
    has_bias = bool(np.any(bqkv))

    nc = _get_nc(Cp, Cq, has_bias)

    in_maps = []
    for core in range(N_CORES):
        b, g = core // 2, core % 2
        u = idx[b]
        n = len(u)
        xk = np.zeros((Cp, D), dtype=np.float32)
        xk[:n] = x[b, u]
        keep = np.zeros(Cp, dtype=np.float32)
        keep[:n] = 1.0
        qs, ks, vs = g * FQ, D + g * FQ, 2 * D + g * FQ
        wqk = np.concatenate(
            [Wqkv[:, qs:qs + FQ], Wqkv[:, ks:ks + FQ]], axis=1
        )
        im = {
            "xT": np.ascontiguousarray(xk.T).astype(bf),
            "wqk": np.ascontiguousarray(wqk).astype(bf),
            "wv": np.ascontiguousarray(Wqkv[:, vs:vs + FQ]).astype(bf),
            "wp": np.ascontiguousarray(Wproj[g * FQ:(g + 1) * FQ, :]).astype(bf),
            "keep": np.ascontiguousarray(keep.reshape(NC, 128).T),
            "onesf": np.ones((1, 64), dtype=np.float32),
        }
        if has_bias:
            bqk = np.concatenate([bqkv[qs:qs + FQ], bqkv[ks:ks + FQ]])
            im["bqkT"] = np.ascontiguousarray(bqk.reshape(8, 128).T)
            im["bvb"] = np.broadcast_to(
                bqkv[vs:vs + FQ], (128, FQ)).astype(np.float32).copy()
        in_maps.append(im)

    global _last_in_maps
    _last_in_maps = in_maps
    res = run_bass_kernel_spmd(nc, in_maps, core_ids=list(range(N_CORES)))

    out = np.broadcast_to(bproj, (B, C, D)).copy()
    for b in range(B):
        u = idx[b]
        n = len(u)
        comb = res.results[2 * b]["outT"] + res.results[2 * b + 1]["outT"]
        out[b, u] += comb.T[:n]
    return out
